# revision 1
# baseline (speedup 1.0000x reference)
"""DCGRU cell Trainium2 kernel (8 NeuronCores, data-parallel over batch).

Per core (4 batches): token tables in HBM (fp16), SpMMs via padded-CSR
"step-k" dma_gather + DVE per-edge scale + identity-matmul PSUM accumulation.
Per-support degree-sort permutations bound padding; all reorders are
host-computed gather indices. The gate gconv post-projects (264-elem tokens,
768B gathers); the cand gconv pre-projects through cand_w (256-elem tokens,
512B gathers). Chebyshev "-x0" terms and the factor 2 fold into weights/values.
"""
import sys

sys.path.insert(0, '/opt/trn_rl_repo')

import numpy as np

N = 10000
U = 64
DIN = 2
B = 32
NCORE = 8
BL = B // NCORE          # 4
F = DIN + U              # 66
NM = 5
TILE = 128
NT = (N + TILE - 1) // TILE   # 79
NPAD = NT * TILE              # 10112
SLAB = 32                     # gather slots per dma_gather
E384 = 384                    # gate-token elems (264 used)
W264 = BL * F                 # 264
E256 = 256                    # cand-token elems
CHUNK = 4                     # token tiles per load/store chunk
GT = 16                       # tiles per idx/v load group

_CACHE = {}


# ----------------------------------------------------------------- host prep
def _support_plan(rows, cols, vals):
    deg = np.zeros(N, np.int64)
    np.add.at(deg, rows, 1)
    pi = np.argsort(deg, kind='stable')
    pipos = np.empty(N, np.int64)
    pipos[pi] = np.arange(N)
    order = np.argsort(pipos[rows], kind='stable')
    srt_pos = pipos[rows][order]
    srt_src = cols[order]
    srt_val = vals[order]
    row_start = np.searchsorted(srt_pos, np.arange(N))
    row_end = np.searchsorted(srt_pos, np.arange(N) + 1)

    d_list, idx_sl, val_sl = [], [], []
    for t in range(NT):
        ps = np.arange(t * TILE, min((t + 1) * TILE, N))
        degs = row_end[ps] - row_start[ps]
        d_t = max(int(degs.max()) if len(ps) else 0, 1)
        it = np.zeros((d_t, TILE), np.int64)
        vt = np.zeros((d_t, TILE), np.float32)
        for r_local, p in enumerate(ps):
            s, e = row_start[p], row_end[p]
            it[:e - s, r_local] = srt_src[s:e]
            vt[:e - s, r_local] = srt_val[s:e]
        d_list.append(d_t)
        idx_sl.append(it)
        val_sl.append(vt)
    return pi, pipos, d_list, np.concatenate(idx_sl, 0), np.concatenate(val_sl, 0)


def _pack_edge_idx(idx_slotmajor):
    flat = idx_slotmajor.reshape(-1).astype(np.int16)
    S = idx_slotmajor.shape[0]
    i = np.arange(S * TILE)
    buf = np.zeros((128, S * 8), np.int16)
    for g in range(8):
        buf[(i % 16) + 16 * g, i // 16] = flat
    return buf


def _pack_tok_idx(vec):
    flat = np.asarray(vec, np.int16)
    M = len(flat)
    assert M % 16 == 0
    i = np.arange(M)
    buf = np.zeros((128, M // 16), np.int16)
    for g in range(8):
        buf[(i % 16) + 16 * g, i // 16] = flat
    return buf


def _pack_vals(val_slotmajor):
    return np.ascontiguousarray(val_slotmajor.T.astype(np.float16))


def _b_slices(width_cols, per_b):
    pieces = []   # (chunk, row0, row1, b, f0)
    for ch in range((width_cols + 127) // 128):
        c0, c1 = 128 * ch, min(128 * (ch + 1), width_cols)
        c = c0
        while c < c1:
            b, f = divmod(c, per_b)
            n = min(c1 - c, per_b - f)
            pieces.append((ch, c - c0, c - c0 + n, b, f))
            c += n
    return pieces


SL66 = _b_slices(W264, F)


def _host_plan(inputs):
    r0 = np.asarray(inputs['s0_rows']); c0 = np.asarray(inputs['s0_cols'])
    w0 = np.asarray(inputs['s0_vals'], np.float32)
    r1 = np.asarray(inputs['s1_rows']); c1 = np.asarray(inputs['s1_cols'])
    w1 = np.asarray(inputs['s1_vals'], np.float32)

    pi0, pipos0, d0, i0, v0 = _support_plan(r0, c0, w0)
    pi1, pipos1, d1, i1, v1 = _support_plan(r1, c1, w1)

    plan = dict(d0=d0, d1=d1, pi0=pi0)
    bufs = {}
    bufs['e0_nat'] = _pack_edge_idx(i0)
    bufs['e0_pos'] = _pack_edge_idx(pipos0[i0])
    bufs['e1_nat'] = _pack_edge_idx(i1)
    bufs['e1_pos0'] = _pack_edge_idx(pipos0[i1])
    bufs['e1_pos1'] = _pack_edge_idx(pipos1[i1])
    bufs['v0'] = _pack_vals(v0)
    bufs['v02'] = _pack_vals(2.0 * v0)
    bufs['v1'] = _pack_vals(v1)
    bufs['v12'] = _pack_vals(2.0 * v1)

    pad0 = np.zeros(NPAD - N, np.int64)
    bufs['tpi0'] = _pack_tok_idx(np.concatenate([pi0, pad0]))
    bufs['tq01'] = _pack_tok_idx(np.concatenate([pipos1[pi0], pad0]))
    bufs['tq10'] = _pack_tok_idx(np.concatenate([pipos0[pi1], pad0]))
    scat = np.concatenate([pi0, np.full(NPAD - N, -1, np.int64)])
    bufs['scat'] = _pack_tok_idx(scat)

    # scatter valid counts per CHUNK of tiles
    valid = []
    nchunk = (NT + CHUNK - 1) // CHUNK
    for c in range(nchunk):
        base = c * TILE * CHUNK
        nidx = min(CHUNK, NT - c * CHUNK) * TILE
        valid.append(int(np.clip(N - base, 0, nidx)))
    plan['scat_valid'] = valid

    # weights: blocks by m with Chebyshev folds
    gwr = np.asarray(inputs['gate_w'], np.float32).reshape(F, NM, 2 * U)
    gB = [gwr[:, 0] - gwr[:, 2] - gwr[:, 4], gwr[:, 1], gwr[:, 2],
          gwr[:, 3], gwr[:, 4]]
    bufs['gw'] = np.concatenate(gB, 1).astype(np.float16)
    cwr = np.asarray(inputs['cand_w'], np.float32).reshape(F, NM, U)
    cB = [cwr[:, 0] - cwr[:, 2] - cwr[:, 4], cwr[:, 1], 2.0 * cwr[:, 2],
          cwr[:, 3], 2.0 * cwr[:, 4]]
    bufs['cw'] = np.concatenate(cB, 1).astype(np.float16)
    bufs['biasg'] = np.tile(np.asarray(inputs['gate_b'], np.float32)[None, :],
                            (128, 1))
    bufs['ident'] = np.eye(128, dtype=np.float16)
    plan['shared_bufs'] = bufs
    return plan


# ------------------------------------------------------------- device program
def _build_program(plan, upto='all', dbg=None):
    import concourse.bacc as bacc
    import concourse.bass as bass
    import concourse.mybir as mybir
    from concourse.tile import TileContext
    from concourse.library_config import mlp

    f16 = mybir.dt.float16
    f32 = mybir.dt.float32
    i16 = mybir.dt.int16
    MUL = mybir.AluOpType.mult
    ADD = mybir.AluOpType.add
    SUB = mybir.AluOpType.subtract
    SIG = mybir.ActivationFunctionType.Sigmoid
    TANH = mybir.ActivationFunctionType.Tanh

    d0, d1 = plan['d0'], plan['d1']
    S0, S1 = sum(d0), sum(d1)
    TOKC = NPAD // 16

    nc = bacc.Bacc('TRN2', target_bir_lowering=False, debug=False)

    x_in = nc.dram_tensor('x_in', [BL, N, DIN], f32, kind='ExternalInput')
    x_st = nc.dram_tensor('x_st', [BL, N, U], f32, kind='ExternalInput')
    ident_d = nc.dram_tensor('ident', [128, 128], f16, kind='ExternalInput')
    biasg_d = nc.dram_tensor('biasg', [128, 2 * U], f32, kind='ExternalInput')
    gw_d = nc.dram_tensor('gw', [F, NM * 2 * U], f16, kind='ExternalInput')
    cw_d = nc.dram_tensor('cw', [F, NM * U], f16, kind='ExternalInput')
    e_d = {k: nc.dram_tensor(k, [128, s * 8], i16, kind='ExternalInput')
           for k, s in (('e0_nat', S0), ('e0_pos', S0), ('e1_nat', S1),
                        ('e1_pos0', S1), ('e1_pos1', S1))}
    v_d = {k: nc.dram_tensor(k, [128, s], f16, kind='ExternalInput')
           for k, s in (('v0', S0), ('v02', S0), ('v1', S1), ('v12', S1))}
    tok_d = {k: nc.dram_tensor(k, [128, TOKC], i16, kind='ExternalInput')
             for k in ('tpi0', 'tq01', 'tq10', 'scat')}
    out_d = nc.dram_tensor('out', [BL, N, U], f32, kind='ExternalOutput')
    _order = ['p0', 'pA', 'pB', 'pC', 'pD', 'rg', 'gate', 'p7', 'p8',
              'p9', 'p10', 'p11']
    _act = set(_order) if upto == 'all' else set(
        _order[:_order.index(upto) + 1])
    dbg_d = (nc.dram_tensor('dbg', [NPAD, E384], f16,
                            kind='ExternalOutput') if dbg else None)

    def scratch(name, shape, dt=f16):
        return nc.dram_tensor(name, shape, dt)

    x0nat = scratch('x0nat', [NPAD, E384])
    x0pi0 = scratch('x0pi0', [NPAD, E384])
    tok1s0 = scratch('tok1s0', [NPAD, E384])
    tok1s1 = scratch('tok1s1', [NPAD, E384])
    tok2s1 = scratch('tok2s1', [NPAD, E384])
    xt = {m: scratch(f'xt{m}', [BL, 68, NPAD]) for m in range(NM)}
    y = {m: scratch(f'y{m}', [NPAD, E256]) for m in range(NM)}
    z0 = scratch('z0', [NPAD, E256])
    z1 = scratch('z1', [NPAD, E256])
    u1t = scratch('u1t', [NPAD, E256])
    v1t = scratch('v1t', [NPAD, E256])
    ug = scratch('ug', [NPAD, E256])

    with TileContext(nc) as tc:
        with (
            tc.tile_pool(name='gp', bufs=2) as gp,
            tc.tile_pool(name='iop', bufs=2) as iop,
            tc.tile_pool(name='tokp', bufs=3) as tokp,
            tc.tile_pool(name='bsp', bufs=2) as bsp,
            tc.tile_pool(name='lhp', bufs=2) as lhp,
            tc.tile_pool(name='misc', bufs=2) as misc,
            tc.tile_pool(name='cst', bufs=1) as cst,
            tc.tile_pool(name='psA', bufs=2, space='PSUM') as psA,
            tc.tile_pool(name='psB', bufs=2, space='PSUM') as psB,
            tc.tile_pool(name='psC', bufs=2, space='PSUM') as psC,
            tc.tile_pool(name='psD', bufs=2, space='PSUM') as psD,
        ):
            nc.gpsimd.load_library(mlp)

            ident = cst.tile([128, 128], f16, name='ident')
            nc.sync.dma_start(ident[:], ident_d[:])
            biasg = cst.tile([128, 2 * U], f32, name='biasg')
            nc.sync.dma_start(biasg[:], biasg_d[:])
            gw = cst.tile([F, NM * 2 * U], f16, name='gw')
            nc.sync.dma_start(gw[:], gw_d[:])
            cw = cst.tile([F, NM * U], f16, name='cw')
            nc.sync.dma_start(cw[:], cw_d[:])
            tok_sb = {}
            for k in ('tpi0', 'tq01', 'tq10', 'scat'):
                tok_sb[k] = cst.tile([128, TOKC], i16, name='tok_' + k)
                nc.sync.dma_start(tok_sb[k][:], tok_d[k][:])

            def bcast(ap, n):
                return bass.AP(ap.tensor, ap.offset, list(ap.ap) + [[0, n]])

            # ---- P0: build x0nat (cast f32->f16)
            if 'p0' in _act:
                for b in range(BL):
                    nc.gpsimd.dma_start(x0nat[0:N, b * F:b * F + DIN], x_in[b])
                    nc.gpsimd.dma_start(x0nat[0:N, b * F + DIN:(b + 1) * F], x_st[b])

            def transpose_to_btables(tok_ap, col0, bstiles):
                for b in range(BL):
                    tp = psB.tile([128, 128], f16, name='tr', tag='tr')
                    nc.tensor.transpose(
                        tp[0:F, :], tok_ap[:, b * F:(b + 1) * F], ident[:])
                    nc.scalar.copy(
                        bstiles[b][0:F, col0:col0 + TILE], tp[0:F, :])

            # ---- generic SpMM
            def spmm(table, elem, dlist, idx_d, vv_d, out_cb):
                width = W264 if elem == E384 else E256
                goff = 0
                for g0 in range(0, NT, GT):
                    tl = list(range(g0, min(g0 + GT, NT)))
                    gs = sum(dlist[t] for t in tl)
                    idx_sb = iop.tile([128, gs * 8], i16, name='idx', tag='idx')
                    v_sb = iop.tile([128, gs], f16, name='val', tag='val')
                    nc.sync.dma_start(
                        idx_sb[:, 0:gs * 8],
                        idx_d[:, goff * 8:(goff + gs) * 8])
                    nc.sync.dma_start(v_sb[:, 0:gs], vv_d[:, goff:goff + gs])
                    off = 0
                    for t in tl:
                        d = dlist[t]
                        ps = psA.tile([128, width], f32, name='sp', tag='sp')
                        for s in range((d + SLAB - 1) // SLAB):
                            k0 = s * SLAB
                            ds = min(SLAB, d - k0)
                            g = gp.tile([128, SLAB, elem], f16, name='G', tag='G')
                            nc.gpsimd.dma_gather(
                                g[:, 0:ds, 0:elem], table[:],
                                idx_sb[:, (off + k0) * 8:(off + k0 + ds) * 8],
                                128 * ds, 128 * ds, elem, single_packet=False)
                            nc.vector.tensor_tensor(
                                g[:, 0:ds, 0:width], g[:, 0:ds, 0:width],
                                bcast(v_sb[:, off + k0:off + k0 + ds], width),
                                op=MUL)
                            for k in range(ds):
                                nc.tensor.matmul(
                                    ps[:], ident[:], g[:, k, 0:width],
                                    start=(k0 + k == 0), stop=(k0 + k == d - 1))
                        out_cb(t, ps)
                        off += d
                    goff += gs

            # ---- P1: x1s0 = S0 x0 (pi0)
            import os
            _simple = bool(os.environ.get('KSIMPLE'))

            def p1_cb(t, ps):
                tok = tokp.tile([128, W264], f16, name='tok', tag='tok')
                nc.scalar.copy(tok[:], ps[:])
                nc.sync.dma_start(tok1s0[t * TILE:(t + 1) * TILE, 0:W264], tok[:])
                if _simple:
                    return
                bst = [bsp.tile([68, TILE], f16, name=f'bs{b}', tag=f'bs{b}') for b in range(BL)]
                transpose_to_btables(tok[:], 0, bst)
                for b in range(BL):
                    nc.sync.dma_start(
                        xt[1][b, 0:68, t * TILE:(t + 1) * TILE], bst[b][:])

            if 'pA' in _act:
                spmm(x0nat, E384, d0, e_d['e0_nat'], v_d['v0'], p1_cb)

            # ---- P2: x2s0~ = 2 S0 x1s0 (pi0)
            def p2_cb(t, ps):
                tok = tokp.tile([128, W264], f16, name='tok', tag='tok')
                nc.scalar.copy(tok[:], ps[:])
                bst = [bsp.tile([68, TILE], f16, name=f'bs{b}', tag=f'bs{b}') for b in range(BL)]
                transpose_to_btables(tok[:], 0, bst)
                for b in range(BL):
                    nc.sync.dma_start(
                        xt[2][b, 0:68, t * TILE:(t + 1) * TILE], bst[b][:])


            # ---- P3: x1s1 = S1 x0 (pi1)
            def p3_cb(t, ps):
                tok = tokp.tile([128, W264], f16, name='tok', tag='tok')
                nc.scalar.copy(tok[:], ps[:])
                nc.sync.dma_start(tok1s1[t * TILE:(t + 1) * TILE, 0:W264], tok[:])

            if 'pC' in _act:
                spmm(x0nat, E384, d1, e_d['e1_nat'], v_d['v1'], p3_cb)
            if 'pB' in _act:
                spmm(tok1s0, E384, d0, e_d['e0_pos'], v_d['v02'], p2_cb)

            # ---- P4: x2s1~ = 2 S1 x1s1 (pi1)
            def p4_cb(t, ps):
                tok = tokp.tile([128, W264], f16, name='tok', tag='tok')
                nc.scalar.copy(tok[:], ps[:])
                nc.sync.dma_start(tok2s1[t * TILE:(t + 1) * TILE, 0:W264], tok[:])


            # ---- P0.5/P5a: token regathers -> x0pi0 + transposed tables
            def regather(src_tab, idx_sb, xtm, tok_out):
                for base in range(0, NPAD, TILE * CHUNK):
                    nt = min(CHUNK, (NPAD - base) // TILE)
                    nidx = TILE * nt
                    g = gp.tile([128, CHUNK, E384], f16, name='G', tag='G')
                    nc.gpsimd.dma_gather(
                        g[:, 0:nt, :], src_tab[:],
                        idx_sb[:, base // 16:(base + nidx) // 16],
                        nidx, nidx, E384, single_packet=False)
                    if tok_out is not None:
                        nc.sync.dma_start(
                            tok_out[base:base + nidx, :].rearrange(
                                '(s r) e -> r s e', r=TILE), g[:, 0:nt, :])
                    if xtm is not None:
                        bst = [bsp.tile([68, TILE * CHUNK], f16, name=f'bc{b}', tag=f'bc{b}')
                               for b in range(BL)]
                        for st in range(nt):
                            transpose_to_btables(g[:, st, 0:W264], st * TILE, bst)
                        for b in range(BL):
                            nc.sync.dma_start(
                                xtm[b, 0:68, base:base + nidx],
                                bst[b][:, 0:nidx])

            if 'rg' in _act:
                regather(x0nat, tok_sb['tpi0'], xt[0], x0pi0)
                regather(tok1s1, tok_sb['tq01'], xt[3], None)
            if 'pD' in _act:
                spmm(tok1s1, E384, d1, e_d['e1_pos1'], v_d['v12'], p4_cb)
            if 'rg' in _act:
                regather(tok2s1, tok_sb['tq01'], xt[4], None)


            # ---- P5/P6: gate matmul + sigmoid + x' + y_m
            SUP = 4
            for sup in (range((NT + SUP - 1) // SUP) if 'gate' in _act else []):
                t0 = sup * SUP
                ntl = min(SUP, NT - t0)
                cols = ntl * TILE
                lh = {}
                for m in range(NM):
                    for b in range(BL):
                        lt = lhp.tile([68, SUP * TILE], f16, name=f'lh{m}{b}', tag=f'lh{m}{b}')
                        nc.sync.dma_start(
                            lt[:, 0:cols],
                            xt[m][b, 0:68, t0 * TILE:t0 * TILE + cols])
                        lh[(m, b)] = lt
                for st in range(ntl):
                    t = t0 + st
                    x0t = misc.tile([128, W264], f16, name='x0t', tag='x0t')
                    nc.sync.dma_start(
                        x0t[:], x0pi0[t * TILE:(t + 1) * TILE, 0:W264])
                    psg = psC.tile([128, BL, 2 * U], f32, name='psg', tag='psg')
                    for b in range(BL):
                        for m in range(NM):
                            nc.tensor.matmul(
                                psg[:, b, :],
                                lh[(m, b)][0:F, st * TILE:(st + 1) * TILE],
                                gw[:, m * 2 * U:(m + 1) * 2 * U],
                                start=(m == 0), stop=(m == NM - 1))
                    xp = misc.tile([128, W264], f16, name='xp', tag='xp')
                    ut = misc.tile([128, BL, U], f16, name='ut', tag='ut')
                    gt = misc.tile([128, BL, 2 * U], f32, name='gt', tag='gt')
                    for b in range(BL):
                        nc.vector.tensor_tensor(
                            psg[:, b, :], psg[:, b, :], biasg[:], op=ADD)
                        nc.scalar.activation(gt[:, b, :], psg[:, b, :], SIG)
                        nc.scalar.copy(ut[:, b, :], gt[:, b, U:2 * U])
                        nc.scalar.copy(
                            xp[:, b * F:b * F + DIN], x0t[:, b * F:b * F + DIN])
                        nc.vector.tensor_tensor(
                            xp[:, b * F + DIN:(b + 1) * F], gt[:, b, 0:U],
                            x0t[:, b * F + DIN:(b + 1) * F], op=MUL)
                    nc.sync.dma_start(ug[t * TILE:(t + 1) * TILE, :], ut[:])
                    xpt = [misc.tile([68, TILE], f16, name=f'xpt{b}', tag=f'xpt{b}')
                           for b in range(BL)]
                    transpose_to_btables(xp[:], 0, xpt)
                    ytok = {m: misc.tile([128, BL, U], f16, name=f'yt{m}', tag=f'yt{m}')
                            for m in range(NM)}
                    for b in range(BL):
                        for m in range(NM):
                            psy = psD.tile([128, U], f32, name='psy', tag='psy')
                            nc.tensor.matmul(
                                psy[:], xpt[b][0:F, :],
                                cw[:, m * U:(m + 1) * U],
                                start=True, stop=True)
                            nc.scalar.copy(ytok[m][:, b, :], psy[:])
                    for m in range(NM):
                        nc.sync.dma_start(
                            y[m][t * TILE:(t + 1) * TILE, :], ytok[m][:])

            # ---- P7: u2 = S0 y2 (pi0); z0 = y1 + u2
            def p7_cb(t, ps):
                y1tl = misc.tile([128, E256], f16, name='y1t', tag='y1t')
                nc.sync.dma_start(y1tl[:], y[1][t * TILE:(t + 1) * TILE, :])
                zt = tokp.tile([128, E256], f16, name='ztok', tag='ztok')
                nc.vector.tensor_tensor(zt[:], ps[:], y1tl[:], op=ADD)
                nc.sync.dma_start(z0[t * TILE:(t + 1) * TILE, :], zt[:])

            if 'p7' in _act:
                spmm(y[2], E256, d0, e_d['e0_pos'], v_d['v0'], p7_cb)

            # ---- P8: u1 = S0 z0 (pi0)
            def p8_cb(t, ps):
                tok = tokp.tile([128, E256], f16, name='ztok', tag='ztok')
                nc.scalar.copy(tok[:], ps[:])
                nc.sync.dma_start(u1t[t * TILE:(t + 1) * TILE, :], tok[:])


            # ---- P9: v2 = S1 y4 (pi1); z1 = y3[q10] + v2
            def p9_cb(t, ps):
                y3t = gp.tile([128, 1, E256], f16, name='Gy', tag='Gy')
                nc.gpsimd.dma_gather(
                    y3t[:, 0:1, :], y[3][:],
                    tok_sb['tq10'][:, t * 8:(t + 1) * 8], TILE, TILE, E256)
                zt = tokp.tile([128, E256], f16, name='ztok', tag='ztok')
                nc.vector.tensor_tensor(zt[:], ps[:], y3t[:, 0, :], op=ADD)
                nc.sync.dma_start(z1[t * TILE:(t + 1) * TILE, :], zt[:])

            if 'p9' in _act:
                spmm(y[4], E256, d1, e_d['e1_pos0'], v_d['v1'], p9_cb)
            if 'p8' in _act:
                spmm(z0, E256, d0, e_d['e0_pos'], v_d['v0'], p8_cb)

            # ---- P10: v1 = S1 z1 (pi1)
            def p10_cb(t, ps):
                tok = tokp.tile([128, E256], f16, name='ztok', tag='ztok')
                nc.scalar.copy(tok[:], ps[:])
                nc.sync.dma_start(v1t[t * TILE:(t + 1) * TILE, :], tok[:])

            if 'p10' in _act:
                spmm(z1, E256, d1, e_d['e1_pos1'], v_d['v1'], p10_cb)

            # ---- P11: mix + tanh + GRU + scatter
            valid = plan['scat_valid']
            for c in (range((NT + CHUNK - 1) // CHUNK) if 'p11' in _act else []):
                base = c * TILE * CHUNK
                nt = min(CHUNK, NT - c * CHUNK)
                nidx = nt * TILE
                y0t = misc.tile([128, CHUNK, E256], f16, name='y0t', tag='y0t')
                u1tt = misc.tile([128, CHUNK, E256], f16, name='u1tt', tag='u1tt')
                ugt = misc.tile([128, CHUNK, E256], f16, name='ugt', tag='ugt')
                x0tt = misc.tile([128, CHUNK, E384], f16, name='x0tt', tag='x0tt')
                v1g = gp.tile([128, CHUNK, E256], f16, name='G', tag='G')
                for (tt, tab) in ((y0t, y[0]), (u1tt, u1t), (ugt, ug)):
                    nc.sync.dma_start(
                        tt[:, 0:nt, :],
                        tab[base:base + nidx, :].rearrange(
                            '(s r) e -> r s e', r=TILE))
                nc.sync.dma_start(
                    x0tt[:, 0:nt, :],
                    x0pi0[base:base + nidx, :].rearrange(
                        '(s r) e -> r s e', r=TILE))
                nc.gpsimd.dma_gather(
                    v1g[:, 0:nt, :], v1t[:],
                    tok_sb['tq01'][:, base // 16:(base + nidx) // 16],
                    nidx, nidx, E256, single_packet=False)
                sc = [misc.tile([128, CHUNK, U], f32, name=f'sc{b}', tag=f'sc{b}')
                      for b in range(BL)]
                for st in range(nt):
                    cd = misc.tile([128, BL, U], f32, name='cd', tag='cd')
                    nc.vector.tensor_tensor(
                        cd[:],
                        y0t[:, st, :].rearrange('p (b u) -> p b u', b=BL),
                        u1tt[:, st, :].rearrange('p (b u) -> p b u', b=BL),
                        op=ADD)
                    nc.vector.tensor_tensor(
                        cd[:], cd[:],
                        v1g[:, st, :].rearrange('p (b u) -> p b u', b=BL),
                        op=ADD)
                    nc.scalar.activation(cd[:], cd[:], TANH)
                    xa = x0tt[:, st, :]
                    sta = bass.AP(xa.tensor, xa.offset + DIN,
                                  [list(xa.ap[0]), [F, BL], [1, U]])
                    dd = misc.tile([128, BL, U], f32, name='dd', tag='dd')
                    nc.vector.tensor_tensor(dd[:], sta, cd[:], op=SUB)
                    nc.vector.tensor_tensor(
                        dd[:], dd[:],
                        ugt[:, st, :].rearrange('p (b u) -> p b u', b=BL),
                        op=MUL)
                    nc.vector.tensor_tensor(cd[:], cd[:], dd[:], op=ADD)
                    for b in range(BL):
                        nc.vector.tensor_copy(sc[b][:, st, :], cd[:, b, :])
                for b in range(BL):
                    nc.gpsimd.dma_scatter_add(
                        out_d[b], sc[b][:, 0:nt, :],
                        tok_sb['scat'][:, base // 16:(base + nidx) // 16],
                        nidx, valid[c], U)

            if dbg:
                _tabs = dict(x0nat=(x0nat, E384), x0pi0=(x0pi0, E384),
                             tok1s0=(tok1s0, E384), tok1s1=(tok1s1, E384),
                             tok2s1=(tok2s1, E384), z0=(z0, E256),
                             z1=(z1, E256), u1t=(u1t, E256),
                             v1t=(v1t, E256), ug=(ug, E256),
                             y0=(y[0], E256), y1=(y[1], E256),
                             y2=(y[2], E256), y3=(y[3], E256),
                             y4=(y[4], E256),
                             xt0=(xt[0], None), xt1=(xt[1], None),
                             xt2=(xt[2], None), xt3=(xt[3], None),
                             xt4=(xt[4], None))
                _tb, _w = _tabs[dbg]
                if _w is None:
                    nc.gpsimd.dma_start(
                        dbg_d[:].rearrange('(x p) e -> x p e', p=68)[0:BL],
                        _tb[:, :, 0:E384])
                else:
                    nc.gpsimd.dma_start(dbg_d[:, 0:_w], _tb[:, 0:_w])
    nc.compile()
    return nc


# ------------------------------------------------------------------ kernel()
def kernel(**inputs):
    from concourse.bass_utils import run_bass_kernel_spmd

    key = 'prog'
    if key not in _CACHE:
        plan = _host_plan(inputs)
        nc = _build_program(plan)
        _CACHE[key] = (plan, nc)
    plan, nc = _CACHE[key]

    inp = np.asarray(inputs['inputs'], np.float32).reshape(B, N, DIN)
    st = np.asarray(inputs['state'], np.float32).reshape(B, N, U)

    shared = dict(plan['shared_bufs'])
    in_maps = []
    for c in range(NCORE):
        m = dict(shared)
        m['x_in'] = np.ascontiguousarray(inp[c * BL:(c + 1) * BL])
        m['x_st'] = np.ascontiguousarray(st[c * BL:(c + 1) * BL])
        in_maps.append(m)

    res = run_bass_kernel_spmd(nc, in_maps, core_ids=list(range(NCORE)))
    out = np.concatenate([r['out'].reshape(BL, N * U) for r in res.results], 0)
    return (out, out)

